# revision 1
# baseline (speedup 1.0000x reference)
"""Trainium2 Bass kernel for relative-position multi-head attention.

Math (derived from the reference, verified numerically):
  The (L,L,depth) relative tensors ak/av are rank-2 in [sin,cos] positional
  features, and the skew unroll is ak[i,j] = a[j-i+L-1].  With
  c = 1.5708/L, sin/cos addition formulas give:

    scores[i,j] = qh_b[i]·kh[j] + A[i]·sin(cj) + B[i]·cos(cj) (+ row-const)
      A = u0*cos_i + u1*sin_i,  B = u1*cos_i - u0*sin_i,  u = qh_b @ Wak^T
    (row-constant terms - including all k-side biases - cancel in softmax)

    out2[i] = P'[i]*Wav[0] + Q'[i]*Wav[1] + bav,   with
      P' = cos_i*Ss - sin_i*Sc,  Q' = cos_i*Sc + sin_i*Ss,
      Ss/Sc = attn-weighted sums of sin_j/cos_j  (extra value columns)

  so the whole relative machinery is +2 contraction rows on the QK matmul
  and +5 value columns on the PV matmul, plus a tiny rank-2 PSUM-accumulated
  correction.  bv/bav/bd fold into one output bias vector; softmax needs no
  max-subtraction (|scores/64| < ~1).  mish(z) = z·(1 - 2/(1+(1+e^z)^2)).

Sharding: data-parallel, no collectives.  Core ci handles batch ci//2 and
query-half ci%2 (512 queries), all 16 heads, and writes its own output rows.
Compute: bf16 matmuls (fp32 PSUM accumulation), fp32 softmax-normalizer path.
Inputs are pre-cast to bf16 on the host; transposed layouts come from
DMA-transpose loads (2-byte dtype requirement satisfied by bf16).
"""

import numpy as np

B, L, D, H, DEPTH = 4, 1024, 1024, 16, 64
IH = 512            # queries per core
CFREQ = 1.5708 / L  # positional frequency (reference uses literal 1.5708)
NCORES = 8

_PROGRAM_CACHE = {}


def _build_program():
    import concourse.bacc as bacc
    import concourse.mybir as mybir
    import concourse.tile as tile
    from contextlib import ExitStack

    f32 = mybir.dt.float32
    f32r = mybir.dt.float32r
    bf16 = mybir.dt.bfloat16
    AF = mybir.ActivationFunctionType
    Alu = mybir.AluOpType

    nc = bacc.Bacc("TRN2", target_bir_lowering=False, debug=False)

    # ---- DRAM I/O ----
    q_d = nc.dram_tensor("q", (IH, D), bf16, kind="ExternalInput")
    k_d = nc.dram_tensor("k", (L, D), bf16, kind="ExternalInput")
    x_d = nc.dram_tensor("x", (L, D), bf16, kind="ExternalInput")
    wq_d = nc.dram_tensor("Wq", (D, D), bf16, kind="ExternalInput")
    wk_d = nc.dram_tensor("Wk", (D, D), bf16, kind="ExternalInput")
    wv_d = nc.dram_tensor("Wv", (D, D), bf16, kind="ExternalInput")
    wd_d = nc.dram_tensor("Wd", (D, D), bf16, kind="ExternalInput")
    bq_d = nc.dram_tensor("bq_cols", (128, 8), f32, kind="ExternalInput")
    g4_d = nc.dram_tensor("G4", (64, 4), bf16, kind="ExternalInput")
    wav_d = nc.dram_tensor("Wav_t", (2, 64), bf16, kind="ExternalInput")
    trow_d = nc.dram_tensor("trig_row2", (2, L), bf16, kind="ExternalInput")
    tq4_d = nc.dram_tensor("trigq4", (4, IH), f32, kind="ExternalInput")
    tp4_d = nc.dram_tensor("trigP4", (4, IH), f32, kind="ExternalInput")
    tcols_d = nc.dram_tensor("trig_cols_rep", (128, 8 * 64), bf16, kind="ExternalInput")
    ones42_d = nc.dram_tensor("ones42", (4, 2), bf16, kind="ExternalInput")
    ones64_d = nc.dram_tensor("ones1x64", (1, 64), f32r, kind="ExternalInput")
    cv_d = nc.dram_tensor("cv128", (128, D), f32, kind="ExternalInput")
    out_d = nc.dram_tensor("out", (IH, D), f32, kind="ExternalOutput")

    VSLOT = 100  # per-head value cols: 64 v | ones@64 (pad) | sin cos sin cos @96..99

    with tile.TileContext(nc) as tc, ExitStack() as top:
        # ---- persistent small constants ----
        cpool = top.enter_context(tc.tile_pool(name="consts", bufs=1))
        g4_t = cpool.tile([64, 4], bf16)
        nc.sync.dma_start(g4_t[:], g4_d.ap())
        wav_t = cpool.tile([2, 64], bf16)
        nc.sync.dma_start(wav_t[:], wav_d.ap())
        trow_t = cpool.tile([2, L], bf16)
        nc.sync.dma_start(trow_t[:], trow_d.ap())
        tq4_t = cpool.tile([4, IH], f32)
        nc.sync.dma_start(tq4_t[:], tq4_d.ap())
        tp4_t = cpool.tile([4, IH], f32)
        nc.sync.dma_start(tp4_t[:], tp4_d.ap())
        tcols_t = cpool.tile([128, 8 * 64], bf16)
        nc.sync.dma_start(tcols_t[:], tcols_d.ap())
        ones42_t = cpool.tile([4, 2], bf16)
        nc.sync.dma_start(ones42_t[:], ones42_d.ap())
        ones64_t = cpool.tile([1, 64], f32r)
        nc.sync.dma_start(ones64_t[:], ones64_d.ap())
        bq_t = cpool.tile([128, 8], f32)
        nc.sync.dma_start(bq_t[:], bq_d.ap())

        # ---- persistent activation/aug tiles ----
        aug_pool = top.enter_context(tc.tile_pool(name="aug", bufs=1))
        k_aug = [aug_pool.tile([66, L], bf16, name=f"k_aug{h}") for h in range(H)]
        q_aug = [aug_pool.tile([66, IH], bf16, name=f"q_aug{h}") for h in range(H)]
        v_int = [aug_pool.tile([128, H * VSLOT], bf16, name=f"v_int{jb}") for jb in range(8)]
        oh_pair = [aug_pool.tile([128, IH], bf16, name=f"oh{p}") for p in range(8)]

        # =========== Phase B: transposed loads + projections ===========
        def transposed_load(dram, n_l, name, trpool):
            "DMA-transpose chunks: tiles[dc] (128, n_l*128) = X[:, dc-block]^T"
            tiles = [trpool.tile([128, n_l * 128], bf16, name=f"{name}T{dc}") for dc in range(8)]
            for dc in range(8):
                nc.sync.dma_start(tiles[dc][:], dram.ap()[:, dc * 128:(dc + 1) * 128],
                                  transpose=True)
            return tiles

        with ExitStack() as phb:
            wpool = phb.enter_context(tc.tile_pool(name="wts", bufs=8))
            prps = phb.enter_context(tc.tile_pool(name="prps", bufs=3, space="PSUM"))
            smps = phb.enter_context(tc.tile_pool(name="smps", bufs=2, space="PSUM"))
            abps = phb.enter_context(tc.tile_pool(name="abps", bufs=2, space="PSUM"))
            tmp_pool = phb.enter_context(tc.tile_pool(name="abtmp", bufs=2))

            # ---- q: load + project + build q_aug ----
            with tc.tile_pool(name="qT", bufs=1) as qTpool:
                qT = transposed_load(q_d, 4, "q", qTpool)
                wq_t = [wpool.tile([128, D], bf16, tag="w", name=f"wq{dc}") for dc in range(8)]
                for dc in range(8):
                    nc.sync.dma_start(wq_t[dc][:], wq_d.ap()[dc * 128:(dc + 1) * 128, :])
                for eb in range(8):
                    ps = prps.tile([128, IH], f32, tag="prj")
                    for dc in range(8):
                        nc.tensor.matmul(
                            ps[:], wq_t[dc][:, eb * 128:(eb + 1) * 128], qT[dc][:],
                            start=(dc == 0), stop=(dc == 7))
                    for s in range(2):  # the two heads in this e-block
                        h = 2 * eb + s
                        nc.vector.tensor_scalar_add(
                            q_aug[h][0:64, :], ps[64 * s:64 * s + 64, :],
                            bq_t[64 * s:64 * s + 64, eb:eb + 1])
                # A/B rows via u4 matmul + elementwise + ones42 matmul
                for h in range(H):
                    u4 = smps.tile([4, IH], f32, tag="u4")
                    nc.tensor.matmul(u4[:], g4_t[:], q_aug[h][0:64, :], start=True, stop=True)
                    t4 = tmp_pool.tile([4, IH], bf16, tag="t4")
                    nc.vector.tensor_mul(t4[:], u4[:], tq4_t[:])
                    ab = abps.tile([2, IH], f32, tag="ab")
                    nc.tensor.matmul(ab[:], ones42_t[:], t4[:], start=True, stop=True)
                    nc.vector.tensor_copy(q_aug[h][64:66, :], ab[:])

            # ---- k: load + project + build k_aug ----
            with tc.tile_pool(name="kT", bufs=1) as kTpool:
                kT = transposed_load(k_d, 8, "k", kTpool)
                wk_t = [wpool.tile([128, D], bf16, tag="w", name=f"wk{dc}") for dc in range(8)]
                for dc in range(8):
                    nc.sync.dma_start(wk_t[dc][:], wk_d.ap()[dc * 128:(dc + 1) * 128, :])
                for eb in range(8):
                    for nh in range(2):
                        ps = prps.tile([128, 512], f32, tag="prj")
                        for dc in range(8):
                            nc.tensor.matmul(
                                ps[:],
                                wk_t[dc][:, eb * 128:(eb + 1) * 128],
                                kT[dc][:, nh * 512:(nh + 1) * 512],
                                start=(dc == 0), stop=(dc == 7))
                        for s in range(2):
                            h = 2 * eb + s
                            nc.vector.tensor_copy(
                                k_aug[h][0:64, nh * 512:(nh + 1) * 512],
                                ps[64 * s:64 * s + 64, :])
                for h in range(H):
                    nc.vector.tensor_copy(k_aug[h][64:66, :], trow_t[:])

            # ---- x: load + project v + build v_int ----
            with tc.tile_pool(name="xT", bufs=1) as xTpool:
                xT = transposed_load(x_d, 8, "x", xTpool)
                wv_t = [wpool.tile([128, D], bf16, tag="w", name=f"wv{dc}") for dc in range(8)]
                for dc in range(8):
                    nc.sync.dma_start(wv_t[dc][:], wv_d.ap()[dc * 128:(dc + 1) * 128, :])
                for jb in range(8):
                    nc.vector.memset(v_int[jb][:], 1.0)
                    vi3 = v_int[jb][:].rearrange("p (h c) -> p h c", h=H)
                    for nh in range(2):
                        ps = prps.tile([128, 512], f32, tag="prj")
                        for dc in range(8):
                            nc.tensor.matmul(
                                ps[:],
                                xT[dc][:, jb * 128:(jb + 1) * 128],
                                wv_t[dc][:, nh * 512:(nh + 1) * 512],
                                start=(dc == 0), stop=(dc == 7))
                        nc.vector.tensor_copy(
                            vi3[:, 8 * nh:8 * nh + 8, 0:64],
                            ps[:].rearrange("p (h c) -> p h c", h=8))
                    nc.vector.tensor_copy(
                        vi3[:, :, 96:100],
                        tcols_t[:, jb * 64:(jb + 1) * 64].rearrange("p (h c) -> p h c", h=H))

        # ====== Phase C: attention, 2 heads at a time, SW-pipelined ======
        with ExitStack() as phc:
            spsum = phc.enter_context(tc.tile_pool(name="spsum", bufs=2, space="PSUM"))
            opsum = phc.enter_context(tc.tile_pool(name="opsum", bufs=2, space="PSUM"))
            pqps = phc.enter_context(tc.tile_pool(name="pqps", bufs=1, space="PSUM"))
            rps = phc.enter_context(tc.tile_pool(name="rps", bufs=1, space="PSUM"))
            epool = phc.enter_context(tc.tile_pool(name="etile", bufs=3))
            cor_pool = phc.enter_context(tc.tile_pool(name="cor", bufs=2))

            for hp in range(8):
                h0, h1 = 2 * hp, 2 * hp + 1
                o_ps = [opsum.tile([VSLOT, IH], f32, tag="ops", name=f"o{h}") for h in (h0, h1)]

                def s_mm(jb):
                    s2 = spsum.tile([128, 2 * IH], f32, tag="s2")
                    for s, h in enumerate((h0, h1)):
                        nc.tensor.matmul(
                            s2[:, s * IH:(s + 1) * IH],
                            k_aug[h][:, jb * 128:(jb + 1) * 128], q_aug[h][:],
                            start=True, stop=True)
                    return s2

                def pv_mm(e2, jb):
                    for s, h in enumerate((h0, h1)):
                        nc.tensor.matmul(
                            o_ps[s][:],
                            v_int[jb][:, h * VSLOT:(h + 1) * VSLOT],
                            e2[:, s * IH:(s + 1) * IH],
                            start=(jb == 0), stop=(jb == 7))

                # software pipeline: S(jb+1) issues before PV(jb) so the PE
                # never waits on the ACT exp of the current block
                pending = None  # (e2, jb)
                s2 = s_mm(0)
                for jb in range(8):
                    e2 = epool.tile([128, 2 * IH], bf16, tag="e2")
                    nc.scalar.activation(e2[:], s2[:], AF.Exp, scale=1.0 / DEPTH)
                    if jb < 7:
                        s2 = s_mm(jb + 1)
                    if pending is not None:
                        pv_mm(*pending)
                    pending = (e2, jb)
                pv_mm(*pending)

                for s, h in enumerate((h0, h1)):
                    ops = o_ps[s]
                    # P'/Q' correction, accumulated into rows 0:64 on PE
                    t4 = cor_pool.tile([4, IH], bf16, tag="pqt")
                    nc.vector.tensor_mul(t4[:], ops[96:100, :], tp4_t[:])
                    pq = pqps.tile([2, IH], f32, tag="pq")
                    nc.tensor.matmul(pq[:], ones42_t[:], t4[:], start=True, stop=True)
                    pq_sb = cor_pool.tile([2, IH], bf16, tag="pqsb")
                    nc.vector.tensor_copy(pq_sb[:], pq[:])
                    nc.tensor.matmul(ops[0:64, :], wav_t[:], pq_sb[:],
                                     start=False, stop=True, skip_group_check=True)
                    # 1/rowsum, broadcast to 64 partitions via K=1 matmul
                    rinv = cor_pool.tile([1, IH], f32, tag="rinv")
                    nc.vector.reciprocal(rinv[:], ops[64:65, :])
                    rinv_r = cor_pool.tile([1, IH], f32r, tag="rinvr")
                    nc.vector.tensor_copy(rinv_r[:], rinv[:])
                    r64 = rps.tile([64, IH], f32, tag="r64")
                    nc.tensor.matmul(r64[:], ones64_t[:], rinv_r[:], start=True, stop=True)
                    r64_sb = cor_pool.tile([64, IH], f32, tag="r64sb")
                    nc.vector.tensor_copy(r64_sb[:], r64[:])
                    nc.vector.tensor_mul(oh_pair[hp][64 * s:64 * s + 64, :], ops[0:64, :], r64_sb[:])

        # =========== Phase D: output dense + bias + mish ===========
        with ExitStack() as phd:
            wdpool = phd.enter_context(tc.tile_pool(name="wd", bufs=8))
            cvpool = phd.enter_context(tc.tile_pool(name="cv", bufs=1))
            zps = phd.enter_context(tc.tile_pool(name="zps", bufs=2, space="PSUM"))
            mpool = phd.enter_context(tc.tile_pool(name="mish", bufs=2))

            cv_t = cvpool.tile([128, D], f32)
            nc.sync.dma_start(cv_t[:], cv_d.ap())
            wd_t = [wdpool.tile([128, D], bf16, tag="wd", name=f"wd{p}") for p in range(8)]
            for p in range(8):
                nc.sync.dma_start(wd_t[p][:], wd_d.ap()[p * 128:(p + 1) * 128, :])

            for ib in range(4):
                z = zps.tile([128, D], f32, tag="z")
                for p in range(8):
                    for nh in range(2):
                        nc.tensor.matmul(
                            z[:, nh * 512:(nh + 1) * 512],
                            oh_pair[p][:, ib * 128:(ib + 1) * 128],
                            wd_t[p][:, nh * 512:(nh + 1) * 512],
                            start=(p == 0), stop=(p == 7))
                zb = mpool.tile([128, D], f32, tag="zb")
                nc.vector.tensor_add(zb[:], z[:], cv_t[:])
                # mish(z) = z * (1 - 2/(1 + (1+e^z)^2))
                ez = mpool.tile([128, D], f32, tag="ez")
                nc.scalar.activation(ez[:], zb[:], AF.Exp)
                sq = mpool.tile([128, D], f32, tag="sq")
                nc.scalar.activation(sq[:], ez[:], AF.Square, bias=1.0)
                den = mpool.tile([128, D], f32, tag="den")
                nc.vector.tensor_scalar_add(den[:], sq[:], 1.0)
                rec = mpool.tile([128, D], f32, tag="rec")
                nc.vector.reciprocal(rec[:], den[:])
                w = mpool.tile([128, D], f32, tag="wmul")
                nc.vector.tensor_scalar(w[:], rec[:], -2.0, 1.0, Alu.mult, Alu.add)
                res = mpool.tile([128, D], f32, tag="res")
                nc.vector.tensor_mul(res[:], zb[:], w[:])
                nc.sync.dma_start(out_d.ap()[ib * 128:(ib + 1) * 128, :], res[:])

    nc.compile()
    return nc


def _host_inputs(x, k, q, Wq, bq, Wk, bk, Wv, bv, Wak, bak, Wav, bav, Wd, bd):
    """Build the per-core input dicts (pure numpy, constant prep only)."""
    import ml_dtypes
    f32 = np.float32
    bf16 = ml_dtypes.bfloat16

    def group(W):  # (H, D, DEPTH) -> (D, H*DEPTH)
        return np.ascontiguousarray(W.transpose(1, 0, 2).reshape(D, H * DEPTH)).astype(bf16)

    W2q, W2k, W2v = group(Wq), group(Wk), group(Wv)
    bq_cols = np.ascontiguousarray(bq.reshape(H * DEPTH).reshape(8, 128).T).astype(f32)

    pos = np.arange(L, dtype=np.float64)
    sin_j = np.sin(CFREQ * pos)
    cos_j = np.cos(CFREQ * pos)
    trig_row2 = np.stack([sin_j, cos_j]).astype(bf16)  # (2, L)

    tcr = np.zeros((128, 8, 16, 4), dtype=np.float64)
    for jb in range(8):
        s = sin_j[jb * 128:(jb + 1) * 128]
        cc = cos_j[jb * 128:(jb + 1) * 128]
        tcr[:, jb, :, 0] = s[:, None]
        tcr[:, jb, :, 1] = cc[:, None]
        tcr[:, jb, :, 2] = s[:, None]
        tcr[:, jb, :, 3] = cc[:, None]
    trig_cols_rep = tcr.reshape(128, 8 * 64).astype(bf16)

    G4 = np.stack([Wak[0], Wak[1], Wak[0], Wak[1]], axis=1).astype(bf16)  # (64, 4)
    Wav_t = np.asarray(Wav, dtype=bf16)  # (2, 64)
    ones42 = np.array([[1, 0], [1, 0], [0, 1], [0, 1]], dtype=bf16)
    ones1x64 = np.ones((1, 64), dtype=f32)

    bhead = (np.asarray(bv, np.float64) + np.asarray(bav, np.float64)[None, :]).reshape(H * DEPTH)
    cvec = bhead @ np.asarray(Wd, np.float64) + np.asarray(bd, np.float64)
    cv128 = np.broadcast_to(cvec.astype(f32), (128, D)).copy()

    in_maps = []
    for ci in range(NCORES):
        b, ih = ci // 2, ci % 2
        i0 = ih * IH
        ii = pos[i0:i0 + IH]
        sin_i, cos_i = np.sin(CFREQ * ii), np.cos(CFREQ * ii)
        trigq4 = np.stack([cos_i, sin_i, -sin_i, cos_i]).astype(f32)   # for A,B
        trigP4 = np.stack([cos_i, -sin_i, sin_i, cos_i]).astype(f32)   # for P',Q'
        in_maps.append({
            "q": np.ascontiguousarray(q[b, i0:i0 + IH]).astype(bf16),
            "k": np.ascontiguousarray(k[b]).astype(bf16),
            "x": np.ascontiguousarray(x[b]).astype(bf16),
            "Wq": W2q, "Wk": W2k, "Wv": W2v,
            "Wd": np.asarray(Wd).astype(bf16),
            "bq_cols": bq_cols,
            "G4": G4, "Wav_t": Wav_t,
            "trig_row2": trig_row2,
            "trigq4": trigq4, "trigP4": trigP4,
            "trig_cols_rep": trig_cols_rep,
            "ones42": ones42, "ones1x64": ones1x64,
            "cv128": cv128,
        })
    return in_maps


def kernel(**inputs):
    from concourse import bass_utils

    x = np.asarray(inputs["x"]); k = np.asarray(inputs["k"]); q = np.asarray(inputs["q"])
    in_maps = _host_inputs(
        x, k, q,
        np.asarray(inputs["Wq"]), np.asarray(inputs["bq"]),
        np.asarray(inputs["Wk"]), np.asarray(inputs["bk"]),
        np.asarray(inputs["Wv"]), np.asarray(inputs["bv"]),
        np.asarray(inputs["Wak"]), np.asarray(inputs["bak"]),
        np.asarray(inputs["Wav"]), np.asarray(inputs["bav"]),
        np.asarray(inputs["Wd"]), np.asarray(inputs["bd"]),
    )
    if "prog" not in _PROGRAM_CACHE:
        _PROGRAM_CACHE["prog"] = _build_program()
    nc = _PROGRAM_CACHE["prog"]
    res = bass_utils.run_bass_kernel_spmd(nc, in_maps, core_ids=list(range(NCORES)))
    out = np.empty((B, L, D), dtype=np.float32)
    for ci in range(NCORES):
        b, ih = ci // 2, ci % 2
        out[b, ih * IH:(ih + 1) * IH, :] = res.results[ci]["out"]
    return out



# revision 26
# speedup vs baseline: 1.2652x; 1.2652x over previous
"""Trainium2 Bass kernel for relative-position multi-head attention.

Math (derived from the reference, validated numerically):
  ak/av are rank-2 in [sin,cos] positional features; the skew unroll gives
  ak[i,j] = a[j-i+L-1].  With c = 1.5708/L:

    scores[i,j] = qh[i]·kh[j] + A[i]·sin(cj) + B[i]·cos(cj) (+ row-consts
      that cancel in softmax).  (A,B) = per-i rotation of u = qh·Wak^T, and
      u is obtained for free by appending host-folded columns (Wq_h @ Wak^T)
      to the Q projection (WU).  So S = k_aug^T q_aug with 66 contract rows.

    value side: per-head value block gets 5 extra columns
      [sin_j, cos_j, sin_j, cos_j, 1] -> PV yields (Ss, Sc, Ss, Sc, rowsum).
      out2 = Wav0·P' + Wav1·Q' with (P',Q') = per-i rotation of (Ss,Sc):
      t4 = (Ss,Sc,Ss,Sc)*(cos,-sin,sin,cos); corr = OW^T t4 with
      OW = [Wav0;Wav0;Wav1;Wav1], accumulated straight into the PV PSUM.

    softmax normalizer: rinv = approx-reciprocal(rowsum), broadcast to 64
    partitions by an SBUF->SBUF DMA with a partition-broadcast AP, applied
    during the PSUM->SBUF move of the head output.

    output dense runs TRANSPOSED (z^T = Wd^T oh) so bias (bv+bav folded
    through Wd, +bd) is per-partition and mish is a single scalar-engine
    Mish activation; the host transposes the [D, IH] result back.

Sharding: data-parallel, no collectives.  Core ci handles batch ci//2 and
query-half ci%2 (512 queries), all 16 heads.  bf16 matmuls, f32 PSUM.
"""

import numpy as np

B, L, D, H, DEPTH = 4, 1024, 1024, 16, 64
IH = 512            # queries per core
CFREQ = 1.5708 / L  # positional frequency (reference uses literal 1.5708)
NCORES = 8
VSLOT = 69          # per-head value cols: 64 v | sin cos sin cos | ones

_PROGRAM_CACHE = {}


def _build_program(debug_dumps=False):
    import concourse.bacc as bacc
    import concourse.mybir as mybir
    import concourse.tile as tile
    from contextlib import ExitStack

    f32 = mybir.dt.float32
    bf16 = mybir.dt.bfloat16
    AF = mybir.ActivationFunctionType

    nc = bacc.Bacc("TRN2", target_bir_lowering=False, debug=False)

    # ---- DRAM I/O ----
    qT_d = nc.dram_tensor("qT", (D, IH), bf16, kind="ExternalInput")
    kT_d = nc.dram_tensor("kT", (D, L), bf16, kind="ExternalInput")
    xT_d = nc.dram_tensor("xT", (D, L), bf16, kind="ExternalInput")
    wq_d = nc.dram_tensor("Wq", (D, D), bf16, kind="ExternalInput")
    wu_d = nc.dram_tensor("WU", (128, 512), bf16, kind="ExternalInput")
    wk_d = nc.dram_tensor("Wk", (D, D), bf16, kind="ExternalInput")
    wv_d = nc.dram_tensor("Wv", (D, D), bf16, kind="ExternalInput")
    wd_d = nc.dram_tensor("Wd", (D, D), bf16, kind="ExternalInput")
    bq_d = nc.dram_tensor("bq_cols", (128, 8), f32, kind="ExternalInput")
    tq64_d = nc.dram_tensor("tq64", (64, IH), f32, kind="ExternalInput")
    m64_d = nc.dram_tensor("M64", (64, 32), bf16, kind="ExternalInput")
    ow_d = nc.dram_tensor("OW", (5, 64), bf16, kind="ExternalInput")
    tp4_d = nc.dram_tensor("tp4", (5, IH), f32, kind="ExternalInput")
    trow_d = nc.dram_tensor("trig_row2", (2, L), bf16, kind="ExternalInput")
    vaux_d = nc.dram_tensor("vaux", (128, 8 * H * 5), bf16, kind="ExternalInput")
    cv_d = nc.dram_tensor("cv_cols", (128, 8), f32, kind="ExternalInput")
    out_d = nc.dram_tensor("out", (D, IH), f32, kind="ExternalOutput")
    if debug_dumps:
        dbg = {
            "dbg_qaug": nc.dram_tensor("dbg_qaug", (66, IH), mybir.dt.bfloat16, kind="ExternalOutput"),
            "dbg_kaug": nc.dram_tensor("dbg_kaug", (66, L), mybir.dt.bfloat16, kind="ExternalOutput"),
            "dbg_vint": nc.dram_tensor("dbg_vint", (128, H * VSLOT), mybir.dt.bfloat16, kind="ExternalOutput"),
            "dbg_oh": nc.dram_tensor("dbg_oh", (128, IH), mybir.dt.bfloat16, kind="ExternalOutput"),
            "dbg_rbc": nc.dram_tensor("dbg_rbc", (128, IH), f32, kind="ExternalOutput"),
            "dbg_ab": nc.dram_tensor("dbg_ab", (32, IH), mybir.dt.bfloat16, kind="ExternalOutput"),
            "dbg_e2": nc.dram_tensor("dbg_e2", (128, 2 * IH), mybir.dt.bfloat16, kind="ExternalOutput"),
            "dbg_e2all": nc.dram_tensor("dbg_e2all", (8 * 128, 2 * IH), mybir.dt.bfloat16, kind="ExternalOutput"),
            "dbg_ops0": nc.dram_tensor("dbg_ops0", (5, IH), f32, kind="ExternalOutput"),
            "dbg_ops1": nc.dram_tensor("dbg_ops1", (5, IH), f32, kind="ExternalOutput"),
            "dbg_ops0post": nc.dram_tensor("dbg_ops0post", (5, IH), f32, kind="ExternalOutput"),
            "dbg_rinv": nc.dram_tensor("dbg_rinv", (2, IH), f32, kind="ExternalOutput"),
        }

    with tile.TileContext(nc) as tc, ExitStack() as top:
        # ---- persistent small constants ----
        cpool = top.enter_context(tc.tile_pool(name="consts", bufs=1))
        trow_t = cpool.tile([2, L], bf16)
        nc.sync.dma_start(trow_t[:], trow_d.ap())
        vaux_t = cpool.tile([128, 8 * H * 5], bf16)
        nc.sync.dma_start(vaux_t[:], vaux_d.ap())
        bq_t = cpool.tile([128, 8], f32)
        nc.sync.dma_start(bq_t[:], bq_d.ap())
        tq64_t = cpool.tile([64, IH], f32)
        nc.sync.dma_start(tq64_t[:], tq64_d.ap())
        m64_t = cpool.tile([64, 32], bf16)
        nc.sync.dma_start(m64_t[:], m64_d.ap())
        ow_t = cpool.tile([5, 64], bf16)
        nc.sync.dma_start(ow_t[:], ow_d.ap())
        tp4_t = cpool.tile([5, IH], f32)
        nc.sync.dma_start(tp4_t[:], tp4_d.ap())
        ones64_t = cpool.tile([1, 64], bf16)
        nc.vector.memset(ones64_t[:], 1.0)
        cv_t = cpool.tile([128, 8], f32)
        nc.sync.dma_start(cv_t[:], cv_d.ap())

        # ---- persistent activation/aug tiles ----
        aug_pool = top.enter_context(tc.tile_pool(name="aug", bufs=1))
        k_aug = [aug_pool.tile([66, L], bf16, name=f"k_aug{h}") for h in range(H)]
        q_aug = [aug_pool.tile([66, IH], bf16, name=f"q_aug{h}") for h in range(H)]
        v_int = [aug_pool.tile([128, H * VSLOT], bf16, name=f"v_int{jb}") for jb in range(8)]
        oh_pair = [aug_pool.tile([128, IH], bf16, name=f"oh{p}") for p in range(8)]

        # weights (wd loaded up-front too; SBUF fits with input pools scoped)
        wdpool = top.enter_context(tc.tile_pool(name="wd", bufs=1))
        wd_t = [wdpool.tile([128, D], bf16, name=f"wd{p}") for p in range(8)]

        # constant rows, written once (off critical path)
        for h in range(H):
            nc.vector.tensor_copy(k_aug[h][64:66, :], trow_t[:])
        for jb in range(8):
            vi3 = v_int[jb][:].rearrange("p (h c) -> p h c", h=H)
            va3 = vaux_t[:, jb * H * 5:(jb + 1) * H * 5].rearrange("p (h c) -> p h c", h=H)
            nc.vector.tensor_copy(vi3[:, :, 64:69], va3)

        # =========== Phase B: projections ===========
        with ExitStack() as phb:
            inq = phb.enter_context(tc.tile_pool(name="inq", bufs=1))
            ink = phb.enter_context(tc.tile_pool(name="ink", bufs=1))
            inx = phb.enter_context(tc.tile_pool(name="inx", bufs=1))
            wqp = phb.enter_context(tc.tile_pool(name="wqp", bufs=1))
            wkp = phb.enter_context(tc.tile_pool(name="wkp", bufs=1))
            wvp = phb.enter_context(tc.tile_pool(name="wvp", bufs=1))
            prps = phb.enter_context(tc.tile_pool(name="prps", bufs=3, space="PSUM"))
            ups = phb.enter_context(tc.tile_pool(name="ups", bufs=1, space="PSUM"))
            tmp_pool = phb.enter_context(tc.tile_pool(name="btmp", bufs=1))

            # DMAs in first-use order
            wq_t = [wqp.tile([128, D], bf16, name=f"wq{dc}") for dc in range(8)]
            for dc in range(8):
                nc.sync.dma_start(wq_t[dc][:], wq_d.ap()[dc * 128:(dc + 1) * 128, :])
            wu_t = wqp.tile([128, 512], bf16, name="wu")
            nc.sync.dma_start(wu_t[:], wu_d.ap())
            qT = [inq.tile([128, IH], bf16, name=f"qT{dc}") for dc in range(8)]
            for dc in range(8):
                nc.sync.dma_start(qT[dc][:], qT_d.ap()[dc * 128:(dc + 1) * 128, :])
            wk_t = [wkp.tile([128, D], bf16, name=f"wk{dc}") for dc in range(8)]
            for dc in range(8):
                nc.sync.dma_start(wk_t[dc][:], wk_d.ap()[dc * 128:(dc + 1) * 128, :])
            kT = [ink.tile([128, L], bf16, name=f"kT{dc}") for dc in range(8)]
            for dc in range(8):
                nc.sync.dma_start(kT[dc][:], kT_d.ap()[dc * 128:(dc + 1) * 128, :])
            wv_t = [wvp.tile([128, D], bf16, name=f"wv{dc}") for dc in range(8)]
            for dc in range(8):
                nc.sync.dma_start(wv_t[dc][:], wv_d.ap()[dc * 128:(dc + 1) * 128, :])
            xT = [inx.tile([128, L], bf16, name=f"xT{dc}") for dc in range(8)]
            for dc in range(8):
                nc.sync.dma_start(xT[dc][:], xT_d.ap()[dc * 128:(dc + 1) * 128, :])
            for p in range(8):
                nc.sync.dma_start(wd_t[p][:], wd_d.ap()[p * 128:(p + 1) * 128, :])

            # ---- Q projection (+ bias into q_aug rows 0:64) ----
            for eb in range(8):
                ps = prps.tile([128, IH], f32, tag="prj")
                for dc in range(8):
                    nc.tensor.matmul(
                        ps[:], wq_t[dc][:, eb * 128:(eb + 1) * 128], qT[dc][:],
                        start=(dc == 0), stop=(dc == 7))
                for s in range(2):
                    h = 2 * eb + s
                    nc.vector.tensor_scalar_add(
                        q_aug[h][0:64, :], ps[64 * s:64 * s + 64, :],
                        bq_t[64 * s:64 * s + 64, eb:eb + 1])

            # ---- U -> (A,B) rows of q_aug ----
            u_ps = ups.tile([64, IH], f32, tag="u64")
            for dc in range(8):
                nc.tensor.matmul(u_ps[:], wu_t[:, dc * 64:(dc + 1) * 64], qT[dc][:],
                                 start=(dc == 0), stop=(dc == 7))
            t64 = tmp_pool.tile([64, IH], bf16, tag="t64")
            nc.vector.tensor_mul(t64[:], u_ps[:], tq64_t[:])
            ab_ps = ups.tile([32, IH], f32, tag="ab")
            nc.tensor.matmul(ab_ps[:], m64_t[:], t64[:], start=True, stop=True)
            ab_sb = tmp_pool.tile([32, IH], bf16, tag="absb")
            nc.vector.tensor_copy(ab_sb[:], ab_ps[:])
            for h in range(H):  # cross-partition move: DMA (engines need 32-aligned base)
                nc.sync.dma_start(q_aug[h][64:66, :], ab_sb[2 * h:2 * h + 2, :])
            if debug_dumps:
                nc.sync.dma_start(dbg["dbg_ab"].ap(), ab_sb[:])

            # ---- K projection ----
            for eb in range(8):
                for nh in range(2):
                    ps = prps.tile([128, 512], f32, tag="prj")
                    for dc in range(8):
                        nc.tensor.matmul(
                            ps[:], wk_t[dc][:, eb * 128:(eb + 1) * 128],
                            kT[dc][:, nh * 512:(nh + 1) * 512],
                            start=(dc == 0), stop=(dc == 7))
                    for s in range(2):
                        h = 2 * eb + s
                        nc.vector.tensor_copy(
                            k_aug[h][0:64, nh * 512:(nh + 1) * 512],
                            ps[64 * s:64 * s + 64, :])

            # ---- V projection ----
            for jb in range(8):
                vi3 = v_int[jb][:].rearrange("p (h c) -> p h c", h=H)
                for nh in range(2):
                    ps = prps.tile([128, 512], f32, tag="prj")
                    for dc in range(8):
                        nc.tensor.matmul(
                            ps[:], xT[dc][:, jb * 128:(jb + 1) * 128],
                            wv_t[dc][:, nh * 512:(nh + 1) * 512],
                            start=(dc == 0), stop=(dc == 7))
                    nc.vector.tensor_copy(
                        vi3[:, 8 * nh:8 * nh + 8, 0:64],
                        ps[:].rearrange("p (h c) -> p h c", h=8))

        # ====== Phase C: attention, 2 heads at a time, SW-pipelined ======
        with ExitStack() as phc:
            spsum = phc.enter_context(tc.tile_pool(name="spsum", bufs=2, space="PSUM"))
            opsum = phc.enter_context(tc.tile_pool(name="opsum", bufs=1, space="PSUM"))
            rps = phc.enter_context(tc.tile_pool(name="rps", bufs=2, space="PSUM"))
            epool = phc.enter_context(tc.tile_pool(name="etile", bufs=3))
            cor_pool = phc.enter_context(tc.tile_pool(name="cor", bufs=2))
            rbc_pool = phc.enter_context(tc.tile_pool(name="rbc", bufs=2))

            for hp in range(8):
                h0, h1 = 2 * hp, 2 * hp + 1
                o_ps = [opsum.tile([VSLOT, IH], f32, tag=f"ops{s}", name=f"o{h}")
                        for s, h in enumerate((h0, h1))]

                def s_mm(jb):
                    s2 = spsum.tile([128, 2 * IH], f32, tag="s2")
                    for s, h in enumerate((h0, h1)):
                        nc.tensor.matmul(
                            s2[:, s * IH:(s + 1) * IH],
                            k_aug[h][:, jb * 128:(jb + 1) * 128], q_aug[h][:],
                            start=True, stop=True)
                    return s2

                def pv_mm(e2, jb):
                    for s, h in enumerate((h0, h1)):
                        nc.tensor.matmul(
                            o_ps[s][:],
                            v_int[jb][:, h * VSLOT:(h + 1) * VSLOT],
                            e2[:, s * IH:(s + 1) * IH],
                            start=(jb == 0), stop=(jb == 7))

                # software pipeline: S(jb+1) issues before PV(jb) so the PE
                # has independent work while ACT computes the exp
                pending = None
                s2 = s_mm(0)
                for jb in range(8):
                    e2 = epool.tile([128, 2 * IH], bf16, tag="e2")
                    nc.scalar.activation(e2[:], s2[:], AF.Exp, scale=1.0 / DEPTH)
                    if debug_dumps and hp == 0 and jb == 0:
                        nc.sync.dma_start(dbg["dbg_e2"].ap(), e2[:])
                    if debug_dumps and hp == 0:
                        nc.sync.dma_start(dbg["dbg_e2all"].ap()[jb * 128:(jb + 1) * 128, :], e2[:])
                    if jb < 7:
                        s2 = s_mm(jb + 1)
                    if pending is not None:
                        pv_mm(*pending)
                    pending = (e2, jb)
                pv_mm(*pending)

                # correction + normalization
                rbc = rbc_pool.tile([128, IH], f32, tag="rbc")
                if debug_dumps and hp == 0:
                    for s in range(2):
                        opsd = cor_pool.tile([5, IH], f32, tag=f"opsd{s}")
                        nc.vector.tensor_copy(opsd[:], o_ps[s][64:69, :])
                        nc.sync.dma_start(dbg[f"dbg_ops{s}"].ap(), opsd[:])
                for s, h in enumerate((h0, h1)):
                    ops = o_ps[s]
                    t4 = cor_pool.tile([5, IH], bf16, tag="t4")
                    nc.vector.tensor_mul(t4[:], ops[64:69, :], tp4_t[:])
                    nc.tensor.matmul(ops[0:64, :], ow_t[:], t4[:],
                                     start=False, stop=True, skip_group_check=True)
                    if debug_dumps and hp == 0 and s == 0:
                        opsp = cor_pool.tile([5, IH], f32, tag="opsp")
                        nc.vector.tensor_copy(opsp[:], ops[64:69, :])
                        nc.sync.dma_start(dbg["dbg_ops0post"].ap(), opsp[:])
                    rs = cor_pool.tile([1, IH], f32, tag="rs")
                    nc.vector.tensor_copy(rs[:], ops[64:65, :])
                    rinv = cor_pool.tile([1, IH], f32, tag="rinv")
                    nc.vector.reciprocal_approx_fast(rinv[:], rs[:])
                    if debug_dumps and hp == 0:
                        nc.sync.dma_start(dbg["dbg_rinv"].ap()[s:s + 1, :], rinv[:])
                    rinv_b = cor_pool.tile([1, IH], bf16, tag="rinvb")
                    nc.vector.tensor_copy(rinv_b[:], rinv[:])
                    r64 = rps.tile([64, IH], f32, tag="r64")
                    nc.tensor.matmul(r64[:], ones64_t[:], rinv_b[:], start=True, stop=True)
                    nc.vector.tensor_copy(rbc[64 * s:64 * s + 64, :], r64[:])
                    nc.vector.tensor_mul(oh_pair[hp][64 * s:64 * s + 64, :],
                                         ops[0:64, :], rbc[64 * s:64 * s + 64, :])
                if debug_dumps and hp == 0:
                    nc.sync.dma_start(dbg["dbg_rbc"].ap(), rbc[:])
                    nc.sync.dma_start(dbg["dbg_oh"].ap(), oh_pair[0][:])
                    nc.sync.dma_start(dbg["dbg_qaug"].ap(), q_aug[0][:])
                    nc.sync.dma_start(dbg["dbg_kaug"].ap(), k_aug[0][:])
                    nc.sync.dma_start(dbg["dbg_vint"].ap(), v_int[0][:])

        # =========== Phase D: transposed output dense + mish ===========
        # mish(z) = z * tanh(ln(1 + e^z)), z tiny here; cv bias folds into
        # the Exp activation and the zb copy.  Exp/Ln/Tanh batched per-func
        # to limit ACT table switches to 2.
        with ExitStack() as phd:
            zps = phd.enter_context(tc.tile_pool(name="zps", bufs=2, space="PSUM"))
            zbp = phd.enter_context(tc.tile_pool(name="zb", bufs=8))
            wp = phd.enter_context(tc.tile_pool(name="wexp", bufs=8))
            spp = phd.enter_context(tc.tile_pool(name="sp", bufs=8))
            thp = phd.enter_context(tc.tile_pool(name="th", bufs=8))
            mpool = phd.enter_context(tc.tile_pool(name="mish", bufs=2))

            zbs, ws, ths = [], [], []
            for dblk in range(8):
                z = zps.tile([128, IH], f32, tag="z")
                for p in range(8):
                    nc.tensor.matmul(
                        z[:], wd_t[p][:, dblk * 128:(dblk + 1) * 128], oh_pair[p][:],
                        start=(p == 0), stop=(p == 7))
                zb = zbp.tile([128, IH], f32, tag="zb")
                nc.vector.tensor_scalar_add(zb[:], z[:], cv_t[:, dblk:dblk + 1])
                w = wp.tile([128, IH], f32, tag="w")
                nc.scalar.activation(w[:], z[:], AF.Exp, bias=cv_t[:, dblk:dblk + 1])
                zbs.append(zb); ws.append(w)
            for dblk in range(8):
                sp = spp.tile([128, IH], f32, tag="sp")
                nc.scalar.activation(sp[:], ws[dblk][:], AF.Ln, bias=1.0)
                ws[dblk] = sp
            for dblk in range(8):
                th = thp.tile([128, IH], f32, tag="th")
                nc.scalar.activation(th[:], ws[dblk][:], AF.Tanh)
                ths.append(th)
            for dblk in range(8):
                res = mpool.tile([128, IH], f32, tag="res")
                nc.vector.tensor_mul(res[:], zbs[dblk][:], ths[dblk][:])
                nc.sync.dma_start(out_d.ap()[dblk * 128:(dblk + 1) * 128, :], res[:])

    nc.compile()
    return nc


def _host_inputs(x, k, q, Wq, bq, Wk, bk, Wv, bv, Wak, bak, Wav, bav, Wd, bd):
    """Build the per-core input dicts (pure numpy, layout/constant prep only)."""
    import ml_dtypes
    f32 = np.float32
    bf16 = ml_dtypes.bfloat16

    def group(W):  # (H, D, DEPTH) -> (D, H*DEPTH)
        return np.ascontiguousarray(W.transpose(1, 0, 2).reshape(D, H * DEPTH)).astype(bf16)

    W2q, W2k, W2v = group(Wq), group(Wk), group(Wv)
    bq_cols = np.ascontiguousarray(bq.reshape(H * DEPTH).reshape(8, 128).T).astype(f32)

    # WU: per head, cols (4h+k) = (Wq_h @ Wak^T) pattern [u0,u1,u0,u1]
    u2 = np.einsum('hde,ke->dhk', np.asarray(Wq, np.float64), np.asarray(Wak, np.float64))
    WU = np.zeros((D, 4 * H))
    WU[:, 0::4] = u2[:, :, 0]
    WU[:, 1::4] = u2[:, :, 1]
    WU[:, 2::4] = u2[:, :, 0]
    WU[:, 3::4] = u2[:, :, 1]
    WU_re = WU.reshape(8, 128, 64).transpose(1, 0, 2).reshape(128, 512).astype(bf16)

    # pairing matrix: A_h = t64[4h]+t64[4h+1], B_h = t64[4h+2]+t64[4h+3]
    M64 = np.zeros((64, 32))
    for h in range(H):
        M64[4 * h + 0, 2 * h] = 1
        M64[4 * h + 1, 2 * h] = 1
        M64[4 * h + 2, 2 * h + 1] = 1
        M64[4 * h + 3, 2 * h + 1] = 1
    M64 = M64.astype(bf16)
    OW = np.stack([np.zeros(64), Wav[0], Wav[0], Wav[1], Wav[1]]).astype(bf16)  # (5, 64)

    pos = np.arange(L, dtype=np.float64)
    sin_j = np.sin(CFREQ * pos)
    cos_j = np.cos(CFREQ * pos)
    trig_row2 = np.stack([sin_j, cos_j]).astype(bf16)  # (2, L)

    # vaux: per jb block, per head: [sin_j, cos_j, sin_j, cos_j, 1]
    va = np.zeros((128, 8, H, 5), dtype=np.float64)
    for jb in range(8):
        s = sin_j[jb * 128:(jb + 1) * 128]
        cc = cos_j[jb * 128:(jb + 1) * 128]
        va[:, jb, :, 0] = 1.0
        va[:, jb, :, 1] = s[:, None]
        va[:, jb, :, 2] = cc[:, None]
        va[:, jb, :, 3] = s[:, None]
        va[:, jb, :, 4] = cc[:, None]
    vaux = va.reshape(128, 8 * H * 5).astype(bf16)

    bhead = (np.asarray(bv, np.float64) + np.asarray(bav, np.float64)[None, :]).reshape(H * DEPTH)
    cvec = bhead @ np.asarray(Wd, np.float64) + np.asarray(bd, np.float64)
    cv_cols = np.ascontiguousarray(cvec.reshape(8, 128).T).astype(f32)

    in_maps = []
    for ci in range(NCORES):
        b, ih = ci // 2, ci % 2
        i0 = ih * IH
        ii = pos[i0:i0 + IH]
        sin_i, cos_i = np.sin(CFREQ * ii), np.cos(CFREQ * ii)
        tq4 = np.stack([cos_i, sin_i, -sin_i, cos_i])
        tq64 = np.tile(tq4, (H, 1)).astype(f32)                       # (64, IH)
        tp4 = np.stack([np.zeros(IH), cos_i, -sin_i, sin_i, cos_i]).astype(f32)  # (5, IH)
        in_maps.append({
            "qT": np.ascontiguousarray(q[b, i0:i0 + IH].T).astype(bf16),
            "kT": np.ascontiguousarray(k[b].T).astype(bf16),
            "xT": np.ascontiguousarray(x[b].T).astype(bf16),
            "Wq": W2q, "Wk": W2k, "Wv": W2v,
            "Wd": np.asarray(Wd).astype(bf16),
            "WU": WU_re,
            "bq_cols": bq_cols,
            "tq64": tq64, "tp4": tp4,
            "M64": M64, "OW": OW,
            "trig_row2": trig_row2,
            "vaux": vaux,
            "cv_cols": cv_cols,
        })
    return in_maps


def kernel(**inputs):
    from concourse import bass_utils

    x = np.asarray(inputs["x"]); k = np.asarray(inputs["k"]); q = np.asarray(inputs["q"])
    in_maps = _host_inputs(
        x, k, q,
        np.asarray(inputs["Wq"]), np.asarray(inputs["bq"]),
        np.asarray(inputs["Wk"]), np.asarray(inputs["bk"]),
        np.asarray(inputs["Wv"]), np.asarray(inputs["bv"]),
        np.asarray(inputs["Wak"]), np.asarray(inputs["bak"]),
        np.asarray(inputs["Wav"]), np.asarray(inputs["bav"]),
        np.asarray(inputs["Wd"]), np.asarray(inputs["bd"]),
    )
    if "prog" not in _PROGRAM_CACHE:
        _PROGRAM_CACHE["prog"] = _build_program()
    nc = _PROGRAM_CACHE["prog"]
    res = bass_utils.run_bass_kernel_spmd(nc, in_maps, core_ids=list(range(NCORES)))
    out = np.empty((B, L, D), dtype=np.float32)
    for ci in range(NCORES):
        b, ih = ci // 2, ci % 2
        out[b, ih * IH:(ih + 1) * IH, :] = res.results[ci]["out"].T
    return out


# revision 30
# speedup vs baseline: 1.7756x; 1.4034x over previous
"""Trainium2 Bass kernel for relative-position multi-head attention.

Math (derived from the reference, validated numerically):
  ak/av are rank-2 in [sin,cos] positional features; the skew unroll gives
  ak[i,j] = a[j-i+L-1].  With c = 1.5708/L:

    scores[i,j] = qh[i]·kh[j] + A[i]·sin(cj) + B[i]·cos(cj) (+ row-consts
      that cancel in softmax).  (A,B) = per-i rotation of u = qh·Wak^T, and
      u is obtained for free by appending host-folded columns (Wq_h @ Wak^T)
      to the Q projection (WU).  So S = k_aug^T q_aug with 66 contract rows.

    value side: per-head value block gets 5 extra columns
      [sin_j, cos_j, sin_j, cos_j, 1] -> PV yields (Ss, Sc, Ss, Sc, rowsum).
      out2 = Wav0·P' + Wav1·Q' with (P',Q') = per-i rotation of (Ss,Sc):
      t4 = (Ss,Sc,Ss,Sc)*(cos,-sin,sin,cos); corr = OW^T t4 with
      OW = [Wav0;Wav0;Wav1;Wav1], accumulated straight into the PV PSUM.

    softmax normalizer: rinv = approx-reciprocal(rowsum), broadcast to 64
    partitions by an SBUF->SBUF DMA with a partition-broadcast AP, applied
    during the PSUM->SBUF move of the head output.

    output dense runs TRANSPOSED (z^T = Wd^T oh) so bias (bv+bav folded
    through Wd, +bd) is per-partition and mish is a single scalar-engine
    Mish activation; the host transposes the [D, IH] result back.

Sharding: data-parallel, no collectives.  Core ci handles batch ci//2 and
query-half ci%2 (512 queries), all 16 heads.  bf16 matmuls, f32 PSUM.
"""

import numpy as np

B, L, D, H, DEPTH = 4, 1024, 1024, 16, 64
IH = 512            # queries per core
CFREQ = 1.5708 / L  # positional frequency (reference uses literal 1.5708)
NCORES = 8
VSLOT = 69          # per-head value cols: 64 v | sin cos sin cos | ones

_PROGRAM_CACHE = {}


def _build_program(debug_dumps=False):
    import concourse.bacc as bacc
    import concourse.mybir as mybir
    import concourse.tile as tile
    from contextlib import ExitStack

    f32 = mybir.dt.float32
    bf16 = mybir.dt.bfloat16
    AF = mybir.ActivationFunctionType

    nc = bacc.Bacc("TRN2", target_bir_lowering=False, debug=False)

    # ---- DRAM I/O ----
    qT_d = nc.dram_tensor("qT", (D, IH), bf16, kind="ExternalInput")
    kT_d = nc.dram_tensor("kT", (D, L), bf16, kind="ExternalInput")
    xT_d = nc.dram_tensor("xT", (D, L), bf16, kind="ExternalInput")
    wq_d = nc.dram_tensor("Wq", (D, D), bf16, kind="ExternalInput")
    wu_d = nc.dram_tensor("WU", (128, 512), bf16, kind="ExternalInput")
    wk_d = nc.dram_tensor("Wk", (D, D), bf16, kind="ExternalInput")
    wv_d = nc.dram_tensor("Wv", (D, D), bf16, kind="ExternalInput")
    wd_d = nc.dram_tensor("Wd", (D, D), bf16, kind="ExternalInput")
    bq_d = nc.dram_tensor("bq_cols", (128, 8), f32, kind="ExternalInput")
    tq64_d = nc.dram_tensor("tq64", (64, IH), f32, kind="ExternalInput")
    m64_d = nc.dram_tensor("M64", (64, 32), bf16, kind="ExternalInput")
    ow_d = nc.dram_tensor("OW", (5, 64), bf16, kind="ExternalInput")
    tp4_d = nc.dram_tensor("tp4", (5, IH), f32, kind="ExternalInput")
    trow_d = nc.dram_tensor("trig_row2", (2, L), bf16, kind="ExternalInput")
    vaux_d = nc.dram_tensor("vaux", (128, 8 * H * 5), bf16, kind="ExternalInput")
    cv_d = nc.dram_tensor("cv_cols", (128, 8), f32, kind="ExternalInput")
    out_d = nc.dram_tensor("out", (D, IH), f32, kind="ExternalOutput")
    if debug_dumps:
        dbg = {
            "dbg_qaug": nc.dram_tensor("dbg_qaug", (66, IH), mybir.dt.bfloat16, kind="ExternalOutput"),
            "dbg_kaug": nc.dram_tensor("dbg_kaug", (66, L), mybir.dt.bfloat16, kind="ExternalOutput"),
            "dbg_vint": nc.dram_tensor("dbg_vint", (128, H * VSLOT), mybir.dt.bfloat16, kind="ExternalOutput"),
            "dbg_oh": nc.dram_tensor("dbg_oh", (128, IH), mybir.dt.bfloat16, kind="ExternalOutput"),
            "dbg_rbc": nc.dram_tensor("dbg_rbc", (128, IH), f32, kind="ExternalOutput"),
            "dbg_ab": nc.dram_tensor("dbg_ab", (32, IH), mybir.dt.bfloat16, kind="ExternalOutput"),
            "dbg_e2": nc.dram_tensor("dbg_e2", (128, 2 * IH), mybir.dt.bfloat16, kind="ExternalOutput"),
            "dbg_e2all": nc.dram_tensor("dbg_e2all", (8 * 128, 2 * IH), mybir.dt.bfloat16, kind="ExternalOutput"),
            "dbg_ops0": nc.dram_tensor("dbg_ops0", (5, IH), f32, kind="ExternalOutput"),
            "dbg_ops1": nc.dram_tensor("dbg_ops1", (5, IH), f32, kind="ExternalOutput"),
            "dbg_ops0post": nc.dram_tensor("dbg_ops0post", (5, IH), f32, kind="ExternalOutput"),
            "dbg_rinv": nc.dram_tensor("dbg_rinv", (2, IH), f32, kind="ExternalOutput"),
        }

    with tile.TileContext(nc) as tc, ExitStack() as top:
        # ---- persistent small constants ----
        cpool = top.enter_context(tc.tile_pool(name="consts", bufs=1))
        trow_t = cpool.tile([2, L], bf16)
        nc.sync.dma_start(trow_t[:], trow_d.ap())
        vaux_t = cpool.tile([128, 8 * H * 5], bf16)
        nc.sync.dma_start(vaux_t[:], vaux_d.ap())
        bq_t = cpool.tile([128, 8], f32)
        nc.sync.dma_start(bq_t[:], bq_d.ap())
        tq64_t = cpool.tile([64, IH], f32)
        nc.sync.dma_start(tq64_t[:], tq64_d.ap())
        m64_t = cpool.tile([64, 32], bf16)
        nc.sync.dma_start(m64_t[:], m64_d.ap())
        ow_t = cpool.tile([5, 64], bf16)
        nc.sync.dma_start(ow_t[:], ow_d.ap())
        tp4_t = cpool.tile([5, IH], f32)
        nc.sync.dma_start(tp4_t[:], tp4_d.ap())
        ones64_t = cpool.tile([1, 64], bf16)
        nc.vector.memset(ones64_t[:], 1.0)
        cv_t = cpool.tile([128, 8], f32)
        nc.sync.dma_start(cv_t[:], cv_d.ap())

        # ---- persistent activation/aug tiles ----
        aug_pool = top.enter_context(tc.tile_pool(name="aug", bufs=1))
        k_aug = [aug_pool.tile([66, L], bf16, name=f"k_aug{h}") for h in range(H)]
        q_aug = [aug_pool.tile([66, IH], bf16, name=f"q_aug{h}") for h in range(H)]
        v_int = [aug_pool.tile([128, H * VSLOT], bf16, name=f"v_int{jb}") for jb in range(8)]
        oh_pair = [aug_pool.tile([128, IH], bf16, name=f"oh{p}") for p in range(8)]

        # weights (wd loaded up-front too; SBUF fits with input pools scoped)
        wdpool = top.enter_context(tc.tile_pool(name="wd", bufs=1))
        wd_t = [wdpool.tile([128, D], bf16, name=f"wd{p}") for p in range(8)]

        # constant rows, written once (off critical path)
        for h in range(H):
            nc.vector.tensor_copy(k_aug[h][64:66, :], trow_t[:])
        for jb in range(8):
            vi3 = v_int[jb][:].rearrange("p (h c) -> p h c", h=H)
            va3 = vaux_t[:, jb * H * 5:(jb + 1) * H * 5].rearrange("p (h c) -> p h c", h=H)
            nc.vector.tensor_copy(vi3[:, :, 64:69], va3)

        # =========== Phase B: projections ===========
        with ExitStack() as phb:
            inq = phb.enter_context(tc.tile_pool(name="inq", bufs=1))
            ink = phb.enter_context(tc.tile_pool(name="ink", bufs=1))
            inx = phb.enter_context(tc.tile_pool(name="inx", bufs=1))
            wqp = phb.enter_context(tc.tile_pool(name="wqp", bufs=1))
            wkp = phb.enter_context(tc.tile_pool(name="wkp", bufs=1))
            wvp = phb.enter_context(tc.tile_pool(name="wvp", bufs=1))
            prps = phb.enter_context(tc.tile_pool(name="prps", bufs=3, space="PSUM"))
            ups = phb.enter_context(tc.tile_pool(name="ups", bufs=1, space="PSUM"))
            tmp_pool = phb.enter_context(tc.tile_pool(name="btmp", bufs=1))

            # DMAs in first-use order
            wq_t = [wqp.tile([128, D], bf16, name=f"wq{dc}") for dc in range(8)]
            for dc in range(8):
                nc.sync.dma_start(wq_t[dc][:], wq_d.ap()[dc * 128:(dc + 1) * 128, :])
            wu_t = wqp.tile([128, 512], bf16, name="wu")
            nc.sync.dma_start(wu_t[:], wu_d.ap())
            qT = [inq.tile([128, IH], bf16, name=f"qT{dc}") for dc in range(8)]
            for dc in range(8):
                nc.sync.dma_start(qT[dc][:], qT_d.ap()[dc * 128:(dc + 1) * 128, :])
            wk_t = [wkp.tile([128, D], bf16, name=f"wk{dc}") for dc in range(8)]
            for dc in range(8):
                nc.sync.dma_start(wk_t[dc][:], wk_d.ap()[dc * 128:(dc + 1) * 128, :])
            kT = [ink.tile([128, L], bf16, name=f"kT{dc}") for dc in range(8)]
            for dc in range(8):
                nc.sync.dma_start(kT[dc][:], kT_d.ap()[dc * 128:(dc + 1) * 128, :])
            wv_t = [wvp.tile([128, D], bf16, name=f"wv{dc}") for dc in range(8)]
            for dc in range(8):
                nc.sync.dma_start(wv_t[dc][:], wv_d.ap()[dc * 128:(dc + 1) * 128, :])
            xT = [inx.tile([128, L], bf16, name=f"xT{dc}") for dc in range(8)]
            for dc in range(8):
                nc.sync.dma_start(xT[dc][:], xT_d.ap()[dc * 128:(dc + 1) * 128, :])
            for p in range(8):
                nc.sync.dma_start(wd_t[p][:], wd_d.ap()[p * 128:(p + 1) * 128, :])

            # ---- Q projection (+ bias into q_aug rows 0:64) ----
            for eb in range(8):
                ps = prps.tile([128, IH], f32, tag="prj")
                for dc in range(8):
                    nc.tensor.matmul(
                        ps[:], wq_t[dc][:, eb * 128:(eb + 1) * 128], qT[dc][:],
                        start=(dc == 0), stop=(dc == 7))
                for s in range(2):
                    h = 2 * eb + s
                    nc.vector.tensor_scalar_add(
                        q_aug[h][0:64, :], ps[64 * s:64 * s + 64, :],
                        bq_t[64 * s:64 * s + 64, eb:eb + 1])

            # ---- U -> (A,B) rows of q_aug ----
            u_ps = ups.tile([64, IH], f32, tag="u64")
            for dc in range(8):
                nc.tensor.matmul(u_ps[:], wu_t[:, dc * 64:(dc + 1) * 64], qT[dc][:],
                                 start=(dc == 0), stop=(dc == 7))
            t64 = tmp_pool.tile([64, IH], bf16, tag="t64")
            nc.vector.tensor_mul(t64[:], u_ps[:], tq64_t[:])
            ab_ps = ups.tile([32, IH], f32, tag="ab")
            nc.tensor.matmul(ab_ps[:], m64_t[:], t64[:], start=True, stop=True)
            ab_sb = tmp_pool.tile([32, IH], bf16, tag="absb")
            nc.vector.tensor_copy(ab_sb[:], ab_ps[:])
            for h in range(H):  # cross-partition move: DMA (engines need 32-aligned base)
                nc.sync.dma_start(q_aug[h][64:66, :], ab_sb[2 * h:2 * h + 2, :])
            if debug_dumps:
                nc.sync.dma_start(dbg["dbg_ab"].ap(), ab_sb[:])

            # ---- K projection ----
            for eb in range(8):
                for nh in range(2):
                    ps = prps.tile([128, 512], f32, tag="prj")
                    for dc in range(8):
                        nc.tensor.matmul(
                            ps[:], wk_t[dc][:, eb * 128:(eb + 1) * 128],
                            kT[dc][:, nh * 512:(nh + 1) * 512],
                            start=(dc == 0), stop=(dc == 7))
                    for s in range(2):
                        h = 2 * eb + s
                        nc.vector.tensor_copy(
                            k_aug[h][0:64, nh * 512:(nh + 1) * 512],
                            ps[64 * s:64 * s + 64, :])

            # ---- V projection ----
            for jb in range(8):
                vi3 = v_int[jb][:].rearrange("p (h c) -> p h c", h=H)
                for nh in range(2):
                    ps = prps.tile([128, 512], f32, tag="prj")
                    for dc in range(8):
                        nc.tensor.matmul(
                            ps[:], xT[dc][:, jb * 128:(jb + 1) * 128],
                            wv_t[dc][:, nh * 512:(nh + 1) * 512],
                            start=(dc == 0), stop=(dc == 7))
                    nc.vector.tensor_copy(
                        vi3[:, 8 * nh:8 * nh + 8, 0:64],
                        ps[:].rearrange("p (h c) -> p h c", h=8))

        # ====== Phase C: attention, 2 heads at a time, SW-pipelined ======
        with ExitStack() as phc:
            spsum = phc.enter_context(tc.tile_pool(name="spsum", bufs=2, space="PSUM"))
            opsum = phc.enter_context(tc.tile_pool(name="opsum", bufs=2, space="PSUM"))
            epool = phc.enter_context(tc.tile_pool(name="etile", bufs=3))
            cor_pool = phc.enter_context(tc.tile_pool(name="cor", bufs=2))
            rbc_pool = phc.enter_context(tc.tile_pool(name="rbc", bufs=2))

            for hp in range(8):
                h0, h1 = 2 * hp, 2 * hp + 1
                o_ps = [opsum.tile([VSLOT, IH], f32, tag=f"ops{s}", name=f"o{h}")
                        for s, h in enumerate((h0, h1))]

                def s_mm(jb):
                    s2 = spsum.tile([128, 2 * IH], f32, tag="s2")
                    for s, h in enumerate((h0, h1)):
                        nc.tensor.matmul(
                            s2[:, s * IH:(s + 1) * IH],
                            k_aug[h][:, jb * 128:(jb + 1) * 128], q_aug[h][:],
                            start=True, stop=True)
                    return s2

                def pv_mm(e2, jb):
                    for s, h in enumerate((h0, h1)):
                        nc.tensor.matmul(
                            o_ps[s][:],
                            v_int[jb][:, h * VSLOT:(h + 1) * VSLOT],
                            e2[:, s * IH:(s + 1) * IH],
                            start=(jb == 0), stop=(jb == 7))

                # software pipeline: S(jb+1) issues before PV(jb) so the PE
                # has independent work while ACT computes the exp
                pending = None
                s2 = s_mm(0)
                for jb in range(8):
                    e2 = epool.tile([128, 2 * IH], bf16, tag="e2")
                    nc.scalar.activation(e2[:], s2[:], AF.Exp, scale=1.0 / DEPTH)
                    if debug_dumps and hp == 0 and jb == 0:
                        nc.sync.dma_start(dbg["dbg_e2"].ap(), e2[:])
                    if debug_dumps and hp == 0:
                        nc.sync.dma_start(dbg["dbg_e2all"].ap()[jb * 128:(jb + 1) * 128, :], e2[:])
                    if jb < 7:
                        s2 = s_mm(jb + 1)
                    if pending is not None:
                        pv_mm(*pending)
                    pending = (e2, jb)
                pv_mm(*pending)

                # correction + normalization
                rbc = rbc_pool.tile([128, IH], bf16, tag="rbc")
                if debug_dumps and hp == 0:
                    for s in range(2):
                        opsd = cor_pool.tile([5, IH], f32, tag=f"opsd{s}")
                        nc.vector.tensor_copy(opsd[:], o_ps[s][64:69, :])
                        nc.sync.dma_start(dbg[f"dbg_ops{s}"].ap(), opsd[:])
                for s, h in enumerate((h0, h1)):
                    ops = o_ps[s]
                    t4 = cor_pool.tile([5, IH], bf16, tag="t4")
                    nc.vector.tensor_mul(t4[:], ops[64:69, :], tp4_t[:])
                    nc.tensor.matmul(ops[0:64, :], ow_t[:], t4[:],
                                     start=False, stop=True, skip_group_check=True)
                    if debug_dumps and hp == 0 and s == 0:
                        opsp = cor_pool.tile([5, IH], f32, tag="opsp")
                        nc.vector.tensor_copy(opsp[:], ops[64:69, :])
                        nc.sync.dma_start(dbg["dbg_ops0post"].ap(), opsp[:])
                    rs = cor_pool.tile([1, IH], f32, tag="rs")
                    nc.vector.tensor_copy(rs[:], ops[64:65, :])
                    rinv = cor_pool.tile([1, IH], f32, tag="rinv")
                    nc.vector.reciprocal_approx_fast(rinv[:], rs[:])
                    if debug_dumps and hp == 0:
                        nc.sync.dma_start(dbg["dbg_rinv"].ap()[s:s + 1, :], rinv[:])
                    rinv_b = cor_pool.tile([1, IH], bf16, tag="rinvb")
                    nc.vector.tensor_copy(rinv_b[:], rinv[:])
                    nc.sync.dma_start(rbc[64 * s:64 * s + 64, :],
                                      rinv_b[:].unsqueeze(1).broadcast_to([1, 64, IH]))
                    nc.vector.tensor_mul(oh_pair[hp][64 * s:64 * s + 64, :],
                                         ops[0:64, :], rbc[64 * s:64 * s + 64, :])
                if debug_dumps and hp == 0:
                    nc.sync.dma_start(dbg["dbg_rbc"].ap(), rbc[:])
                    nc.sync.dma_start(dbg["dbg_oh"].ap(), oh_pair[0][:])
                    nc.sync.dma_start(dbg["dbg_qaug"].ap(), q_aug[0][:])
                    nc.sync.dma_start(dbg["dbg_kaug"].ap(), k_aug[0][:])
                    nc.sync.dma_start(dbg["dbg_vint"].ap(), v_int[0][:])

        # =========== Phase D: transposed output dense + mish ===========
        # mish(z) = z * tanh(ln(1 + e^z)), z tiny here; cv bias folds into
        # the Exp activation and the zb copy.  Exp/Ln/Tanh batched per-func
        # to limit ACT table switches to 2.
        with ExitStack() as phd:
            zps = phd.enter_context(tc.tile_pool(name="zps", bufs=2, space="PSUM"))
            zbp = phd.enter_context(tc.tile_pool(name="zb", bufs=8))
            wp = phd.enter_context(tc.tile_pool(name="wexp", bufs=8))
            spp = phd.enter_context(tc.tile_pool(name="sp", bufs=8))
            thp = phd.enter_context(tc.tile_pool(name="th", bufs=8))
            mpool = phd.enter_context(tc.tile_pool(name="mish", bufs=2))

            # mish(zb) = zb*(s-1)/(s+1), s = (1+e^zb)^2 — Exp/Square only, so
            # the whole kernel uses a single ACT table (no reloads).
            for dblk in range(8):
                z = zps.tile([128, IH], f32, tag="z")
                for p in range(8):
                    nc.tensor.matmul(
                        z[:], wd_t[p][:, dblk * 128:(dblk + 1) * 128], oh_pair[p][:],
                        start=(p == 0), stop=(p == 7))
                zb = zbp.tile([128, IH], f32, tag="zb")
                nc.vector.tensor_scalar_add(zb[:], z[:], cv_t[:, dblk:dblk + 1])
                w = wp.tile([128, IH], f32, tag="w")
                nc.scalar.activation(w[:], z[:], AF.Exp, bias=cv_t[:, dblk:dblk + 1])
                sq = spp.tile([128, IH], f32, tag="sq")
                nc.scalar.activation(sq[:], w[:], AF.Square, bias=1.0)
                t1 = thp.tile([128, IH], f32, tag="t1")
                nc.vector.scalar_tensor_tensor(
                    t1[:], sq[:], -1.0, zb[:],
                    mybir.AluOpType.add, mybir.AluOpType.mult)
                den = thp.tile([128, IH], f32, tag="den")
                nc.vector.tensor_scalar_add(den[:], sq[:], 1.0)
                rden = zbp.tile([128, IH], f32, tag="rden")
                nc.vector.reciprocal_approx_fast(rden[:], den[:])
                res = mpool.tile([128, IH], f32, tag="res")
                nc.vector.tensor_mul(res[:], t1[:], rden[:])
                nc.sync.dma_start(out_d.ap()[dblk * 128:(dblk + 1) * 128, :], res[:])

    nc.compile()
    return nc


def _host_inputs(x, k, q, Wq, bq, Wk, bk, Wv, bv, Wak, bak, Wav, bav, Wd, bd):
    """Build the per-core input dicts (pure numpy, layout/constant prep only)."""
    import ml_dtypes
    f32 = np.float32
    bf16 = ml_dtypes.bfloat16

    def group(W):  # (H, D, DEPTH) -> (D, H*DEPTH)
        return np.ascontiguousarray(W.transpose(1, 0, 2).reshape(D, H * DEPTH)).astype(bf16)

    W2q, W2k, W2v = group(Wq), group(Wk), group(Wv)
    bq_cols = np.ascontiguousarray(bq.reshape(H * DEPTH).reshape(8, 128).T).astype(f32)

    # WU: per head, cols (4h+k) = (Wq_h @ Wak^T) pattern [u0,u1,u0,u1]
    u2 = np.einsum('hde,ke->dhk', np.asarray(Wq, np.float64), np.asarray(Wak, np.float64))
    WU = np.zeros((D, 4 * H))
    WU[:, 0::4] = u2[:, :, 0]
    WU[:, 1::4] = u2[:, :, 1]
    WU[:, 2::4] = u2[:, :, 0]
    WU[:, 3::4] = u2[:, :, 1]
    WU_re = WU.reshape(8, 128, 64).transpose(1, 0, 2).reshape(128, 512).astype(bf16)

    # pairing matrix: A_h = t64[4h]+t64[4h+1], B_h = t64[4h+2]+t64[4h+3]
    M64 = np.zeros((64, 32))
    for h in range(H):
        M64[4 * h + 0, 2 * h] = 1
        M64[4 * h + 1, 2 * h] = 1
        M64[4 * h + 2, 2 * h + 1] = 1
        M64[4 * h + 3, 2 * h + 1] = 1
    M64 = M64.astype(bf16)
    OW = np.stack([np.zeros(64), Wav[0], Wav[0], Wav[1], Wav[1]]).astype(bf16)  # (5, 64)

    pos = np.arange(L, dtype=np.float64)
    sin_j = np.sin(CFREQ * pos)
    cos_j = np.cos(CFREQ * pos)
    trig_row2 = np.stack([sin_j, cos_j]).astype(bf16)  # (2, L)

    # vaux: per jb block, per head: [sin_j, cos_j, sin_j, cos_j, 1]
    va = np.zeros((128, 8, H, 5), dtype=np.float64)
    for jb in range(8):
        s = sin_j[jb * 128:(jb + 1) * 128]
        cc = cos_j[jb * 128:(jb + 1) * 128]
        va[:, jb, :, 0] = 1.0
        va[:, jb, :, 1] = s[:, None]
        va[:, jb, :, 2] = cc[:, None]
        va[:, jb, :, 3] = s[:, None]
        va[:, jb, :, 4] = cc[:, None]
    vaux = va.reshape(128, 8 * H * 5).astype(bf16)

    bhead = (np.asarray(bv, np.float64) + np.asarray(bav, np.float64)[None, :]).reshape(H * DEPTH)
    cvec = bhead @ np.asarray(Wd, np.float64) + np.asarray(bd, np.float64)
    cv_cols = np.ascontiguousarray(cvec.reshape(8, 128).T).astype(f32)

    in_maps = []
    for ci in range(NCORES):
        b, ih = ci // 2, ci % 2
        i0 = ih * IH
        ii = pos[i0:i0 + IH]
        sin_i, cos_i = np.sin(CFREQ * ii), np.cos(CFREQ * ii)
        tq4 = np.stack([cos_i, sin_i, -sin_i, cos_i])
        tq64 = np.tile(tq4, (H, 1)).astype(f32)                       # (64, IH)
        tp4 = np.stack([np.zeros(IH), cos_i, -sin_i, sin_i, cos_i]).astype(f32)  # (5, IH)
        in_maps.append({
            "qT": np.ascontiguousarray(q[b, i0:i0 + IH].T).astype(bf16),
            "kT": np.ascontiguousarray(k[b].T).astype(bf16),
            "xT": np.ascontiguousarray(x[b].T).astype(bf16),
            "Wq": W2q, "Wk": W2k, "Wv": W2v,
            "Wd": np.asarray(Wd).astype(bf16),
            "WU": WU_re,
            "bq_cols": bq_cols,
            "tq64": tq64, "tp4": tp4,
            "M64": M64, "OW": OW,
            "trig_row2": trig_row2,
            "vaux": vaux,
            "cv_cols": cv_cols,
        })
    return in_maps


def kernel(**inputs):
    from concourse import bass_utils

    x = np.asarray(inputs["x"]); k = np.asarray(inputs["k"]); q = np.asarray(inputs["q"])
    in_maps = _host_inputs(
        x, k, q,
        np.asarray(inputs["Wq"]), np.asarray(inputs["bq"]),
        np.asarray(inputs["Wk"]), np.asarray(inputs["bk"]),
        np.asarray(inputs["Wv"]), np.asarray(inputs["bv"]),
        np.asarray(inputs["Wak"]), np.asarray(inputs["bak"]),
        np.asarray(inputs["Wav"]), np.asarray(inputs["bav"]),
        np.asarray(inputs["Wd"]), np.asarray(inputs["bd"]),
    )
    if "prog" not in _PROGRAM_CACHE:
        _PROGRAM_CACHE["prog"] = _build_program()
    nc = _PROGRAM_CACHE["prog"]
    res = bass_utils.run_bass_kernel_spmd(nc, in_maps, core_ids=list(range(NCORES)))
    out = np.empty((B, L, D), dtype=np.float32)
    for ci in range(NCORES):
        b, ih = ci // 2, ci % 2
        out[b, ih * IH:(ih + 1) * IH, :] = res.results[ci]["out"].T
    return out
